# revision 71
# baseline (speedup 1.0000x reference)
"""Trainium2 Bass kernel for 16-head self-attention (N=4, S=2048, E=1024).

Sharding: 8 cores = 4 batches x 2 head-groups (8 heads each).  Each core
computes its head-group's attention and a partial fc_out product
(S x 1024); the host sums the two partials per batch and adds the bias.

Device layout (per core):
  - inputs arrive pre-transposed/cast by the host:
      xqT/xkT/xvT: (4, 128, S) bf16   -- head-dim on partitions, pair-blocks
      wq/wk/wv:    (128, 64) bf16     -- W.T stacked twice (rows 0-63 / 64-127)
      woT:         (4, 128, 1024) bf16 -- Wo.T row-blocks for this head-group
  - projections on PE with diagonal 2-head tile packing
  - scoresT = K.T-blocks (stationary) @ Q.T (moving), 2-head row packing
    (the two head matmuls row-tile into disjoint 64-row PE groups and run
    concurrently)
  - exp on ScalarE (scale=1/sqrt(E) fused), (128,1024) chunks from PSUM;
    the Scalar engine is the throughput floor (~218us of exp per core),
    so everything else is scheduled to keep it continuously fed
  - ctx matmul lhsT = [Vproj | ones] (M=65): row 64 accumulates sumexp
  - normalization: reciprocal_approx_fast (DVE) from the PSUM sum row,
    DRAM partition-broadcast of the reciprocal, fused multiply+cast into
    the bf16 ctxT tile
  - fc_out: bf16 matmuls accumulating over 4 head-pair blocks, interleaved
    into the NEXT query-chunk's attention stream so the PE/Act pipeline
    never drains; output DMAed straight from PSUM to DRAM
"""

import numpy as np
import ml_dtypes

EMBED = 1024
HEADS = 16
HD = 64  # head dim
N_CORES = 8
HPC = 8  # heads per core
GCOLS = HPC * HD  # 512 embed columns per core


def ts(i, n):
    return slice(i * n, (i + 1) * n)


def build_program(S, taps=False):
    import concourse.bass as bass
    import concourse.tile as tile
    import concourse.mybir as mybir
    from concourse import bacc

    f32 = mybir.dt.float32
    bf16 = mybir.dt.bfloat16
    EXP = mybir.ActivationFunctionType.Exp

    NPAIR = 4  # head-pair blocks (2 heads each)
    NJB = S // 128  # key blocks

    nc = bacc.Bacc("TRN2", target_bir_lowering=False, debug=False)

    xqT_d = nc.dram_tensor("xqT", [NPAIR, 128, S], bf16, kind="ExternalInput").ap()
    xkT_d = nc.dram_tensor("xkT", [NPAIR, 128, S], bf16, kind="ExternalInput").ap()
    xvT_d = nc.dram_tensor("xvT", [NPAIR, 128, S], bf16, kind="ExternalInput").ap()
    wq_d = nc.dram_tensor("wq", [128, HD], bf16, kind="ExternalInput").ap()
    wk_d = nc.dram_tensor("wk", [128, HD], bf16, kind="ExternalInput").ap()
    wv_d = nc.dram_tensor("wv", [128, HD], bf16, kind="ExternalInput").ap()
    wo_d = nc.dram_tensor("woT", [NPAIR, 128, EMBED], bf16, kind="ExternalInput").ap()
    out_d = nc.dram_tensor("out", [S, EMBED], f32, kind="ExternalOutput").ap()
    rec_dram = nc.dram_tensor("rec_scratch", [4, 2, S], f32)
    tap_d = {}
    if taps:
        for nm, shape, dt_ in (
            ("dbg_qT", [128, S], bf16),
            ("dbg_kT", [128, S], bf16),
            ("dbg_vp", [128, HPC * 65], bf16),
            ("dbg_exp", [128, min(S, 1024)], bf16),
            ("dbg_ctxT", [128, S], bf16),
            ("dbg_rec", [1, 1024], f32),
            ("dbg_cu", [64, 512], f32),
            ("dbg_rrs", [64, 512], f32),
        ):
            tap_d[nm] = nc.dram_tensor(nm, shape, dt_, kind="ExternalOutput").ap()

    with tile.TileContext(nc) as tc:
        import contextlib

        with contextlib.ExitStack() as ctx:
            # ---- persistent pools ----
            const_p = ctx.enter_context(tc.tile_pool(name="const", bufs=1))
            qkv_p = ctx.enter_context(tc.tile_pool(name="qkv", bufs=1))
            vp_p = ctx.enter_context(tc.tile_pool(name="vp", bufs=1))
            ctxT_p = ctx.enter_context(tc.tile_pool(name="ctxT", bufs=1))
            wo_p = ctx.enter_context(tc.tile_pool(name="wo", bufs=1))
            # PSUM pools: "sc" slots are 2 banks each (x2 bufs = 4 banks),
            # "ctx" slots 1 bank (x4 bufs = 4 banks) -> 8 banks total.
            sc_ps = ctx.enter_context(tc.tile_pool(name="sc", bufs=2, space="PSUM"))
            ctx_ps = ctx.enter_context(tc.tile_pool(name="ctxps", bufs=4, space="PSUM"))

            # ---- constants / weights ----
            wq_s = const_p.tile([128, HD], bf16, tag="wq")
            wk_s = const_p.tile([128, HD], bf16, tag="wk")
            wv_s = const_p.tile([128, HD], bf16, tag="wv")
            nc.sync.dma_start(wq_s[:], wq_d[:])
            nc.sync.dma_start(wk_s[:], wk_d[:])
            nc.sync.dma_start(wv_s[:], wv_d[:])
            wo_t = [wo_p.tile([128, EMBED], bf16, tag=f"wo{p}", name=f"wo{p}") for p in range(NPAIR)]

            qT = [qkv_p.tile([128, S], bf16, tag=f"qT{p}", name=f"qT{p}") for p in range(NPAIR)]
            kT = [qkv_p.tile([128, S], bf16, tag=f"kT{p}", name=f"kT{p}") for p in range(NPAIR)]
            vp_t = [vp_p.tile([128, HPC * 65], bf16, tag=f"vp{jb}", name=f"vp{jb}") for jb in range(NJB)]
            ctxT = [ctxT_p.tile([128, S], bf16, tag=f"cx{p}", name=f"cx{p}") for p in range(NPAIR)]

            # exp pool created early: the startup warm stream below buffers
            # exp outputs for pairs 0-1 so the Act engine starts ~25us sooner
            exp_p = ctx.enter_context(tc.tile_pool(name="exp", bufs=36))
            inv_sqrt_e = 1.0 / float(np.sqrt(EMBED))
            warm_e = {0: [], 1: []}

            with tc.tile_pool(name="xin", bufs=1) as xin_p:
                xq = [xin_p.tile([128, S], bf16, tag=f"xq{p}", name=f"xq{p}") for p in range(NPAIR)]
                xk = [xin_p.tile([128, S], bf16, tag=f"xk{p}", name=f"xk{p}") for p in range(NPAIR)]
                xv = [xin_p.tile([128, S], bf16, tag=f"xv{p}", name=f"xv{p}") for p in range(NPAIR)]
                # spread input slabs over the SP and Act HWDGE queues so
                # transfers run on two DMA paths; wo loads last (needed only
                # from chunk 1's fc_out)
                for p in range(NPAIR):
                    nc.sync.dma_start(xq[p][:], xqT_d[p])
                    nc.scalar.dma_start(xk[p][:], xkT_d[p])
                    (nc.scalar if p % 2 else nc.sync).dma_start(xv[p][:], xvT_d[p])
                for p in range(NPAIR):
                    (nc.sync if p % 2 else nc.scalar).dma_start(wo_t[p][:], wo_d[p])

                # ---- Q/K projections: qT = (W.T).T @ xT, diagonal 2-head pack
                def emit_qk(w_s, x_t, dst, p, ch, pool, tag):
                    ps = pool.tile([128, 512], f32, tag=tag)
                    for b in (0, 64):
                        nc.tensor.matmul(
                            ps[b : b + 64, 0:512],
                            lhsT=w_s[b : b + 64, :],
                            rhs=x_t[p][b : b + 64, ts(ch, 512)],
                            start=True,
                            stop=True,
                        )
                    nc.vector.tensor_copy(dst[p][:, ts(ch, 512)], ps[:, 0:512])

                # minimal projections for pairs 0-1 (q chunk 0 + all k chunks)
                for p in (0, 1):
                    emit_qk(wq_s, xq, qT, p, 0, sc_ps, "sc")
                    for ch in range(4):
                        emit_qk(wk_s, xk, kT, p, ch, sc_ps, "sc")

                # warm stream: scores+exp for pairs 0-1, chunk 0, all key
                # blocks — buffered in exp_p and consumed by the main loop.
                # Remaining projection pieces interleave on the ctx psum pool
                # (independent rotation, so they are not Act-paced).
                rest = [(wq_s, xq, qT, p, ch) for p in (0, 1) for ch in (1, 2, 3)]
                rest += [
                    (w, x, d, p, ch)
                    for p in (2, 3)
                    for (w, x, d) in ((wq_s, xq, qT), (wk_s, xk, kT))
                    for ch in range(4)
                ]
                for pr in (0, 1):
                    for jb in range(NJB):
                        if rest:
                            w, x, d, p, ch = rest.pop(0)
                            emit_qk(w, x, d, p, ch, ctx_ps, "ctx")
                        s_t = sc_ps.tile([128, 1024], f32, tag="sc")
                        for hl, b in ((0, 0), (1, 64)):
                            nc.tensor.matmul(
                                s_t[:, ts(hl, 512)],
                                lhsT=kT[pr][b : b + 64, ts(jb, 128)],
                                rhs=qT[pr][b : b + 64, 0:512],
                                start=True,
                                stop=True,
                            )
                        e_t = exp_p.tile([128, 1024], bf16, tag="exp")
                        nc.scalar.activation(e_t[:], s_t[:], EXP, scale=inv_sqrt_e)
                        warm_e[pr].append(e_t)
                for w, x, d, p, ch in rest:
                    emit_qk(w, x, d, p, ch, ctx_ps, "ctx")

                if taps:
                    nc.sync.dma_start(tap_d["dbg_qT"][:], qT[0][:])
                    nc.sync.dma_start(tap_d["dbg_kT"][:], kT[0][:])

                # ---- V projection into [Vproj | ones] tiles ----
                for jb in range(NJB):
                    pse = ctx_ps.tile([128, 256], f32, tag="ctx")
                    pso = ctx_ps.tile([128, 256], f32, tag="ctx")
                    for h in range(HPC):
                        p, b = h // 2, (h % 2) * 64
                        dst = pse if h % 2 == 0 else pso
                        nc.tensor.matmul(
                            dst[:, ts(h // 2, 64)],
                            lhsT=xv[p][b : b + 64, ts(jb, 128)],
                            rhs=wv_s[b : b + 64, :],
                            start=True,
                            stop=True,
                        )
                    vpr = vp_t[jb].rearrange(
                        "p (h2 two c) -> p h2 two c", two=2, c=65
                    )
                    pse_r = pse.rearrange("p (h c) -> p h c", c=64)
                    pso_r = pso.rearrange("p (h c) -> p h c", c=64)
                    nc.vector.tensor_copy(vpr[:, :, 0, 0:64], pse_r[:])
                    nc.vector.tensor_copy(vpr[:, :, 1, 0:64], pso_r[:])
                    nc.vector.memset(vpr[:, :, :, 64], 1.0)

            if taps:
                nc.sync.dma_start(tap_d["dbg_vp"][:], vp_t[0][:])

            # transient pools (opened after xin frees its SBUF)
            cu_p = ctx.enter_context(tc.tile_pool(name="cu", bufs=6))
            rec_p = ctx.enter_context(tc.tile_pool(name="rec", bufs=4))
            rrs_p = ctx.enter_context(tc.tile_pool(name="rrs", bufs=4))
            fco_p = ctx.enter_context(tc.tile_pool(name="fco", bufs=3))

            # ---- fc_out emission (one 512-col output block) ----
            def emit_fco(sb, oc, eng=None):
                ps = ctx_ps.tile([128, 512], f32, tag="ctx")
                for p in range(NPAIR):
                    nc.tensor.matmul(
                        ps[:],
                        lhsT=ctxT[p][:, ts(sb, 128)],
                        rhs=wo_t[p][:, ts(oc, 512)],
                        start=(p == 0),
                        stop=(p == NPAIR - 1),
                    )
                fo = fco_p.tile([128, 512], f32, tag="fco")
                nc.vector.tensor_copy(fo[:], ps[:])
                (eng or nc.sync).dma_start(out_d[ts(sb, 128), ts(oc, 512)], fo[:])

            # ---- attention main loop (cc outer, pair inner) ----
            # scores tile (128,1024) = [head A i-chunk | head B i-chunk];
            # one exp op covers the pair.  ctx accumulates per (pr, cc) with
            # the fused ones-column providing sumexp in row 64.  fc_out for
            # the previous chunk is drip-fed into this chunk's jb stream so
            # neither PE nor Act ever drains.
            NCCS = S // 512  # 512-wide i-chunks
            for cc in range(NCCS):
                # fc_out work for the previous chunk: 8 groups, drip-fed
                fco_q = []
                if cc > 0:
                    fco_q = [((cc - 1) * 4 + g // 2, g % 2) for g in range(8)]
                gjb = 0
                for pr in range(NPAIR):
                    cpsA = ctx_ps.tile([65, 512], f32, tag="ctx", name=f"cpsA{pr}_{cc}")
                    cpsB = ctx_ps.tile([65, 512], f32, tag="ctx", name=f"cpsB{pr}_{cc}")
                    for jb in range(NJB):
                        if cc == 0 and pr in warm_e:
                            # scores+exp already ran in the startup warm stream
                            e_t = warm_e[pr][jb]
                        else:
                            s_t = sc_ps.tile([128, 1024], f32, tag="sc")
                            for hl, b in ((0, 0), (1, 64)):
                                nc.tensor.matmul(
                                    s_t[:, ts(hl, 512)],
                                    lhsT=kT[pr][b : b + 64, ts(jb, 128)],
                                    rhs=qT[pr][b : b + 64, ts(cc, 512)],
                                    start=True,
                                    stop=True,
                                )
                            e_t = exp_p.tile([128, 1024], bf16, tag="exp")
                            nc.scalar.activation(e_t[:], s_t[:], EXP, scale=inv_sqrt_e)
                        if taps and pr == 0 and cc == 0 and jb == 0:
                            nc.sync.dma_start(tap_d["dbg_exp"][:], e_t[:])
                        for hl, cps in ((0, cpsA), (1, cpsB)):
                            hh = pr * 2 + hl
                            nc.tensor.matmul(
                                cps[:],
                                lhsT=vp_t[jb][:, hh * 65 : hh * 65 + 65],
                                rhs=e_t[:, ts(hl, 512)],
                                start=(jb == 0),
                                stop=(jb == NJB - 1),
                            )
                        # drip one fc_out group every 8th jb iteration
                        if fco_q and gjb % 8 == 4:
                            emit_fco(*fco_q.pop(0))
                        gjb += 1
                    # streamed normalization for this (pr, cc)
                    # gather the two sumexp rows at partition 0 (the custom
                    # DVE reciprocal silently ignores partition offsets), then
                    # one fast reciprocal over both heads
                    se_c = rec_p.tile([1, 1024], f32, tag="se")
                    rec_c = rec_p.tile([1, 1024], f32, tag="rec")
                    cus = []
                    for hl, cps in ((0, cpsA), (1, cpsB)):
                        cu = cu_p.tile([65, 512], f32, tag="cu", name=f"cu{pr}_{cc}_{hl}")
                        nc.vector.tensor_copy(cu[:], cps[:])
                        nc.sync.dma_start(se_c[0:1, ts(hl, 512)], cu[64:65, :])
                        cus.append(cu)
                    nc.vector.reciprocal_approx_fast(rec_c[:], se_c[:])
                    nc.sync.dma_start(rec_dram[pr][:, ts(cc, 512)], rec_c[:])
                    if taps and pr == 0 and cc == 0:
                        nc.sync.dma_start(tap_d["dbg_rec"][:], rec_c[:])
                        nc.sync.dma_start(tap_d["dbg_cu"][:], cus[0][:])
                    for hl in range(2):
                        rrs_c = rrs_p.tile([64, 512], f32, tag="rrs")
                        nc.sync.dma_start(
                            rrs_c[:],
                            rec_dram[pr][hl : hl + 1, ts(cc, 512)].partition_broadcast(64),
                        )
                        if taps and pr == 0 and cc == 0 and hl == 0:
                            nc.sync.dma_start(tap_d["dbg_rrs"][:], rrs_c[:])
                        nc.vector.tensor_mul(
                            ctxT[pr][hl * 64 : hl * 64 + 64, ts(cc, 512)],
                            cus[hl][0:64, :],
                            rrs_c[:],
                        )
                # any fc_out groups not yet drip-fed
                for sb, oc in fco_q:
                    emit_fco(sb, oc)
            # trailing fc_out for the last chunk: the exp stream is over, so
            # split the 2MB output drain across both HWDGE queues (Act idle)
            for g in range(8):
                emit_fco(
                    (NCCS - 1) * 4 + g // 2,
                    g % 2,
                    nc.scalar if g % 2 else nc.sync,
                )

            if taps:
                nc.sync.dma_start(tap_d["dbg_ctxT"][:], ctxT[0][:])

    nc.compile()
    return nc


def make_core_inputs(values, keys, queries, Wv, Wk, Wq, Wo, n, g, S):
    """Host-side marshaling for core (n, g): transpose + cast input slices."""
    bf = ml_dtypes.bfloat16
    cols = slice(g * GCOLS, (g + 1) * GCOLS)
    NPAIR = 4

    def xt(x):
        t = np.ascontiguousarray(x[n][:, cols].T.astype(bf))  # (512, S)
        return t.reshape(NPAIR, 128, S)

    def wstack(w):
        wt = w.T.astype(bf)  # (64, 64)
        return np.ascontiguousarray(np.concatenate([wt, wt], axis=0))  # (128, 64)

    woT = np.ascontiguousarray(Wo[:, cols].T.astype(bf)).reshape(
        NPAIR, 128, EMBED
    )
    return {
        "xqT": xt(queries),
        "xkT": xt(keys),
        "xvT": xt(values),
        "wq": wstack(Wq),
        "wk": wstack(Wk),
        "wv": wstack(Wv),
        "woT": woT,
    }


_PROG_CACHE = {}
TRACE = False
LAST_RESULTS = None


def kernel(values, keys, queries, mask, Wv, Wk, Wq, Wo, bo):
    global LAST_RESULTS
    from concourse.bass_utils import run_bass_kernel_spmd

    values = np.asarray(values, np.float32)
    keys = np.asarray(keys, np.float32)
    queries = np.asarray(queries, np.float32)
    Wv = np.asarray(Wv, np.float32)
    Wk = np.asarray(Wk, np.float32)
    Wq = np.asarray(Wq, np.float32)
    Wo = np.asarray(Wo, np.float32)
    bo = np.asarray(bo, np.float32)

    N, S, _ = queries.shape
    if S not in _PROG_CACHE:
        _PROG_CACHE[S] = build_program(S)
    nc = _PROG_CACHE[S]

    in_maps = [
        make_core_inputs(values, keys, queries, Wv, Wk, Wq, Wo, c // 2, c % 2, S)
        for c in range(N_CORES)
    ]
    res = run_bass_kernel_spmd(
        nc, in_maps, core_ids=list(range(N_CORES)), trace=TRACE
    )
    LAST_RESULTS = res
    out = np.empty((N, S, EMBED), np.float32)
    for n in range(N):
        out[n] = res.results[2 * n]["out"] + res.results[2 * n + 1]["out"] + bo
    return out
